# revision 6
# baseline (speedup 1.0000x reference)
"""Trainium2 Bass kernel: 3D Gaussian mixture rendered on a voxel grid.

Computes grid[z,y,x] = sum_a amp * prod_axis (voxel-averaged 1D gaussian
integrals via erf), i.e. a sum of 2048 separable outer products.

Strategy:
  - Shard the output grid along y: core i renders y-pixels [16i, 16i+16).
    No collectives; host concatenates the 8 disjoint slabs.
  - Host-side atom culling per slab: atoms farther than MARGIN*sigma from
    the slab contribute < 1e-13 relatively and are dropped. ~630 atoms
    survive per slab -> NBLK=5 blocks of 128 atoms (padded, masked).
  - Device per atom-block:
      ACT:  erf at the 129 pixel *edges* per axis (one erf per edge --
            gx[p] = 0.5*(E[p+1]-E[p]) reuses each edge eval twice).
      DVE:  shifted-slice subtractions -> gx [128a,128], gz [128a,128],
            gy [128a,16]; gys = gy * mask * c_amp.
      DVE/ACT: Khatri-Rao H[a,(y,x)] = gx * gys_y (16 broadcast mults).
      PE:   grid[z, (y,x)] += gz.T @ H, accumulated in PSUM over blocks
            (contraction over atoms), float32r for full-rate fp32 matmul.
  - PSUM -> SBUF -> HBM.
"""

import os

import numpy as np

import concourse.bacc as bacc
import concourse.tile as tile
from concourse import mybir
from concourse.bass_utils import run_bass_kernel_spmd

N_PIX = 128
N_CORES = 8
SLAB = N_PIX // N_CORES  # 16 y-pixels per core
NBLK = 5  # atom blocks of 128 per core
CAP = NBLK * 128
MARGIN_SIGMA = 7.5  # cull atoms farther than this (in sigmas) from the slab

# which of the 16 per-block Khatri-Rao ops run on ScalarE instead of VectorE
ACT_YS = frozenset({3, 7, 11, 15})
USE_F32R = True  # float32r matmul: full fp32 storage, full-rate PE

LAST_RESULTS = None  # BassKernelResults of the most recent run (for test.py)


def _build_nc(inv_d: float, c_amp: float):
    f32 = mybir.dt.float32
    f32r = mybir.dt.float32r
    Erf = mybir.ActivationFunctionType.Erf
    mult = mybir.AluOpType.mult

    nc = bacc.Bacc(None, target_bir_lowering=False, name="gauss3d")
    edges_d = nc.dram_tensor("edges", [128, N_PIX + 1], f32, kind="ExternalInput")
    yedges_d = nc.dram_tensor("yedges", [128, SLAB + 1], f32, kind="ExternalInput")
    posx_d = nc.dram_tensor("posx", [128, NBLK], f32, kind="ExternalInput")
    posy_d = nc.dram_tensor("posy", [128, NBLK], f32, kind="ExternalInput")
    posz_d = nc.dram_tensor("posz", [128, NBLK], f32, kind="ExternalInput")
    mask_d = nc.dram_tensor("mask", [128, NBLK], f32, kind="ExternalInput")
    grid_d = nc.dram_tensor("grid", [128, SLAB * N_PIX], f32, kind="ExternalOutput")

    with tile.TileContext(nc) as tc:
        with (
            tc.tile_pool(name="const", bufs=1) as const,
            tc.tile_pool(name="work", bufs=3) as work,
            tc.tile_pool(name="h", bufs=2) as hpool,
            tc.tile_pool(name="o", bufs=2) as opool,
            tc.tile_pool(name="ps", bufs=1, space="PSUM") as psum,
        ):
            edges_t = const.tile([128, N_PIX + 1], f32)
            nc.sync.dma_start(edges_t[:], edges_d[:])
            yedges_t = const.tile([128, SLAB + 1], f32)
            nc.sync.dma_start(yedges_t[:], yedges_d[:])
            posx_t = const.tile([128, NBLK], f32)
            nc.sync.dma_start(posx_t[:], posx_d[:])
            posy_t = const.tile([128, NBLK], f32)
            nc.sync.dma_start(posy_t[:], posy_d[:])
            posz_t = const.tile([128, NBLK], f32)
            nc.sync.dma_start(posz_t[:], posz_d[:])
            mask_t = const.tile([128, NBLK], f32)
            nc.sync.dma_start(mask_t[:], mask_d[:])

            # activation computes func(in*scale + bias): bias_col = -pos*inv_d
            bx = const.tile([128, NBLK], f32)
            nc.vector.tensor_scalar_mul(bx[:], posx_t[:], -inv_d)
            by = const.tile([128, NBLK], f32)
            nc.vector.tensor_scalar_mul(by[:], posy_t[:], -inv_d)
            bz = const.tile([128, NBLK], f32)
            nc.vector.tensor_scalar_mul(bz[:], posz_t[:], -inv_d)

            ps = psum.tile([128, SLAB * N_PIX], f32)

            for b in range(NBLK):
                ex = work.tile([128, N_PIX + 1], f32, tag="ex")
                nc.scalar.activation(ex[:], edges_t[:], Erf, bias=bx[:, b : b + 1], scale=inv_d)
                ez = work.tile([128, N_PIX + 1], f32, tag="ez")
                nc.scalar.activation(ez[:], edges_t[:], Erf, bias=bz[:, b : b + 1], scale=inv_d)
                ey = work.tile([128, SLAB + 1], f32, tag="ey")
                nc.scalar.activation(ey[:], yedges_t[:], Erf, bias=by[:, b : b + 1], scale=inv_d)

                mm_dt = f32r if USE_F32R else f32
                gx = work.tile([128, N_PIX], f32, tag="gx")
                nc.vector.tensor_sub(gx[:], ex[:, 1 : N_PIX + 1], ex[:, 0:N_PIX])
                gz = work.tile([128, N_PIX], mm_dt, tag="gz")
                nc.vector.tensor_sub(gz[:], ez[:, 1 : N_PIX + 1], ez[:, 0:N_PIX])
                gy = work.tile([128, SLAB], f32, tag="gy")
                nc.vector.tensor_sub(gy[:], ey[:, 1 : SLAB + 1], ey[:, 0:SLAB])
                # fold pad-mask and the global amp*(0.5/vs)^3 scale into gy
                gys = work.tile([128, SLAB], f32, tag="gys")
                nc.vector.tensor_scalar(
                    gys[:], gy[:], mask_t[:, b : b + 1], c_amp, mult, mult
                )

                h = hpool.tile([128, SLAB, N_PIX], mm_dt)
                for y in range(SLAB):
                    if y in ACT_YS:
                        nc.scalar.mul(h[:, y, :], gx[:], gys[:, y : y + 1])
                    else:
                        nc.vector.tensor_scalar_mul(h[:, y, :], gx[:], gys[:, y : y + 1])

                lhsT = gz[:]
                for c in range(4):
                    rhs = h[:, 4 * c : 4 * c + 4, :]
                    nc.tensor.matmul(
                        ps[:, 512 * c : 512 * (c + 1)],
                        lhsT=lhsT,
                        rhs=rhs,
                        start=(b == 0),
                        stop=(b == NBLK - 1),
                        skip_group_check=True,
                    )

            for c in range(4):
                ot = opool.tile([128, 512], f32, tag="ot")
                if c % 2 == 0:
                    nc.vector.tensor_copy(ot[:], ps[:, 512 * c : 512 * (c + 1)])
                else:
                    nc.scalar.copy(ot[:], ps[:, 512 * c : 512 * (c + 1)])
                nc.sync.dma_start(grid_d[:, 512 * c : 512 * (c + 1)], ot[:])

    nc.compile()
    return nc


def _shard_inputs(pos: np.ndarray, sigma: float, vs: float, n_pix: int):
    """Per-core input dicts: edge-coordinate tiles + culled/padded atom blocks."""
    edges = ((np.arange(n_pix + 1, dtype=np.float32) - n_pix // 2) - 0.5) * np.float32(vs)
    edge_tile = np.ascontiguousarray(np.tile(edges, (128, 1)), dtype=np.float32)

    w = np.float32(MARGIN_SIGMA * sigma)
    in_maps = []
    for i in range(N_CORES):
        e_lo = edges[SLAB * i]
        e_hi = edges[SLAB * i + SLAB]
        py = pos[:, 1]
        m = (py >= e_lo - w) & (py <= e_hi + w)
        idx = np.nonzero(m)[0]
        if len(idx) > CAP:
            # keep the CAP atoms closest to the slab (farther ones are the
            # ones the margin already proved negligible)
            d = np.maximum(0.0, np.maximum(e_lo - py[idx], py[idx] - e_hi))
            idx = idx[np.argsort(d, kind="stable")[:CAP]]
        n = len(idx)
        p = np.zeros((CAP, 3), dtype=np.float32)
        p[:n] = pos[idx]
        # pads: harmless in-range position; mask kills their contribution
        p[n:, 0] = 0.0
        p[n:, 1] = np.float32((e_lo + e_hi) / 2)
        p[n:, 2] = 0.0
        mask = np.zeros((CAP,), dtype=np.float32)
        mask[:n] = 1.0

        def blk(v):  # [CAP] -> [128, NBLK] (partition = index within block)
            return np.ascontiguousarray(v.reshape(NBLK, 128).T, dtype=np.float32)

        yedge_tile = np.ascontiguousarray(
            np.tile(edges[SLAB * i : SLAB * i + SLAB + 1], (128, 1)), dtype=np.float32
        )
        in_maps.append(
            {
                "edges": edge_tile,
                "yedges": yedge_tile,
                "posx": blk(p[:, 0]),
                "posy": blk(p[:, 1]),
                "posz": blk(p[:, 2]),
                "mask": blk(mask),
            }
        )
    return in_maps


def kernel(
    atom_positions: np.ndarray,
    log_var: np.ndarray,
    log_weight: np.ndarray,
    n_pix,
    voxel_size,
) -> np.ndarray:
    global LAST_RESULTS
    pos = np.asarray(atom_positions, dtype=np.float32)
    lv = float(np.asarray(log_var, dtype=np.float32).reshape(-1)[0])
    lw = float(np.asarray(log_weight, dtype=np.float32).reshape(-1)[0])
    n_pix = int(n_pix)
    vs = float(voxel_size)
    assert n_pix == N_PIX, f"kernel compiled for n_pix={N_PIX}, got {n_pix}"

    sigma = float(np.exp(0.5 * lv))
    amp = float(np.exp(lw))
    inv_d = float(1.0 / (np.sqrt(2.0) * sigma))
    c_amp = float(amp * (0.5 / vs) ** 3)

    in_maps = _shard_inputs(pos, sigma, vs, n_pix)
    nc = _build_nc(inv_d, c_amp)
    res = run_bass_kernel_spmd(
        nc,
        in_maps,
        core_ids=list(range(N_CORES)),
        trace=bool(int(os.environ.get("GAUSS3D_TRACE", "0"))),
    )
    LAST_RESULTS = res
    grids = [r["grid"].reshape(N_PIX, SLAB, N_PIX) for r in res.results]
    return np.ascontiguousarray(np.concatenate(grids, axis=1), dtype=np.float32)
